# revision 1
# baseline (speedup 1.0000x reference)
"""CARAFE (content-aware reassembly of features) Trainium2 Bass kernel.

Problem (hardcoded shapes):
  x       [8, 128, 64, 64] f32
  comp_w  [64, 128, 1, 1]   1x1 conv -> BN(train stats) -> SiLU
  enc_w   [100, 64, 3, 3]   3x3 conv -> BN(train stats)
  pixel_shuffle(2) -> softmax over 25 taps -> weighted 5x5 (dilation 2)
  reassembly of nearest-upsampled x. Output [8, 128, 128, 128] f32.

Sharding: data-parallel over batch, 1 image per core on 8 cores.
BN batch stats are made exact with two tiny AllReduces (sum & sumsq).

Key layout trick: with output pixel (y,x) = (2i+di, 2j+dj) and tap (dy,dx),
the reassembly source is x[c, i+dy-2, j+dx-2] -- independent of (di,dj).
So everything runs at low resolution with shifted views of a zero-padded x;
the pixel-shuffle and nearest-upsample are folded into access patterns.
"""

import sys

import numpy as np

sys.path.insert(0, "/opt/trn_rl_repo")

P = 128          # partitions / input channels
MID = 64         # compressed channels
NENC = 100       # encoder output channels = 25 taps * 4 subpixels
H = W = 64
PX = H * W       # 4096 low-res pixels per image
HP = H + 4       # zero-padded (pad=2) low-res frame for 5x5 dil-2 taps
H1 = H + 2       # zero-padded (pad=1) frame for the 3x3 conv
HM = 2 * H       # 128 upsampled
OUT = HM * HM    # 16384 output pixels per image
NB = 8           # batch / cores
NSTAT = NB * PX  # BN normalization count (N*H*W)
EPS = 1e-5
CHUNK = 512      # free-dim chunk = 8 low-res rows
NCHUNK = PX // CHUNK

_CACHE = {}


def _build_program():
    import concourse.bass as bass
    import concourse.mybir as mybir
    import concourse.tile as tile
    from concourse import bacc

    fp32 = mybir.dt.float32
    bf16 = mybir.dt.bfloat16
    Alu = mybir.AluOpType
    Act = mybir.ActivationFunctionType

    nc = bacc.Bacc(None, num_devices=NB)

    with tile.TileContext(nc) as tc:
        with tc.tile_pool(name="dram", bufs=1, space="DRAM") as dram:
            # I/O
            x_d = dram.tile([P, PX], fp32, kind="ExternalInput", name="x", uniquify=False)
            w1t_d = dram.tile([P, MID], fp32, kind="ExternalInput", name="w1t", uniquify=False)
            w2t_d = dram.tile([MID, 9 * NENC], fp32, kind="ExternalInput", name="w2t", uniquify=False)
            g1_d = dram.tile([MID, 2], fp32, kind="ExternalInput", name="g1b1", uniquify=False)
            g2_d = dram.tile([NENC, 2], fp32, kind="ExternalInput", name="g2b2", uniquify=False)
            sel4_d = dram.tile([NENC, 4], fp32, kind="ExternalInput", name="sel4", uniquify=False)
            sel100_d = dram.tile([4, NENC], fp32, kind="ExternalInput", name="sel100", uniquify=False)
            ones1_d = dram.tile([1, P], fp32, kind="ExternalInput", name="ones1", uniquify=False)
            eye_d = dram.tile([NENC, NENC], fp32, kind="ExternalInput", name="eye100", uniquify=False)
            eye128_d = dram.tile([P, P], fp32, kind="ExternalInput", name="eye128", uniquify=False)
            out_d = dram.tile([P, OUT], fp32, kind="ExternalOutput", name="out", uniquify=False)
            # collective bounce buffers (internal DRAM)
            ar1_in = dram.tile([2, MID], fp32, name="ar1_in")
            ar1_out = dram.tile([2, MID], fp32, name="ar1_out")
            ar2_in = dram.tile([2, NENC], fp32, name="ar2_in")
            ar2_out = dram.tile([2, NENC], fp32, name="ar2_out")

            with (
                tc.tile_pool(name="const", bufs=1) as const,
                tc.tile_pool(name="big", bufs=1) as big,
                tc.tile_pool(name="small", bufs=1) as small,
                tc.tile_pool(name="scratch", bufs=3) as scratch,
                tc.tile_pool(name="tmpp", bufs=3) as tmpp,
                tc.tile_pool(name="ps", bufs=2, space="PSUM") as ps,
            ):
                # ---- load constants ----
                w1t = const.tile([P, MID], fp32)
                nc.sync.dma_start(w1t[:], w1t_d[:])
                w2t = const.tile([MID, 9 * NENC], fp32)
                nc.sync.dma_start(w2t[:], w2t_d[:])
                g1b1 = const.tile([MID, 2], fp32)
                nc.sync.dma_start(g1b1[:], g1_d[:])
                g2b2 = const.tile([NENC, 2], fp32)
                nc.sync.dma_start(g2b2[:], g2_d[:])
                sel4 = const.tile([NENC, 4], fp32)
                nc.sync.dma_start(sel4[:], sel4_d[:])
                sel100 = const.tile([4, NENC], fp32)
                nc.sync.dma_start(sel100[:], sel100_d[:])
                ones1 = const.tile([1, P], fp32)
                nc.sync.dma_start(ones1[:], ones1_d[:])
                eye100 = const.tile([NENC, NENC], fp32)
                nc.sync.dma_start(eye100[:], eye_d[:])
                eye128 = const.tile([P, P], fp32)
                nc.sync.dma_start(eye128[:], eye128_d[:])

                # ---- padded x ----
                xpad = big.tile([P, HP, HP], fp32)
                nc.vector.memset(xpad[:], 0.0)
                nc.sync.dma_start(xpad[:, 2 : 2 + H, 2 : 2 + W], x_d[:].rearrange("p (h w) -> p h w", h=H))


                # ---- conv1 (1x1, 128->64) + stats ----
                y1 = big.tile([MID, PX], fp32, tag="ybuf", name="y1")
                s1c = small.tile([MID, NCHUNK], fp32)
                ss1c = small.tile([MID, NCHUNK], fp32)
                for c in range(NCHUNK):
                    r0 = c * 8
                    pt = ps.tile([P, 2 * CHUNK], fp32, tag="wb", bufs=2, name="pt1")[:MID, :CHUNK]
                    nc.tensor.matmul(
                        pt[:], w1t[:], xpad[:, 2 + r0 : 10 + r0, 2 : 2 + W], start=True, stop=True
                    )
                    nc.scalar.activation(
                        y1[:, c * CHUNK : (c + 1) * CHUNK], pt[:], Act.Copy,
                        accum_out=s1c[:, c : c + 1],
                    )
                    sq = scratch.tile([MID, CHUNK], fp32, tag="sq1")
                    nc.scalar.activation(
                        sq[:], pt[:], Act.Square, accum_out=ss1c[:, c : c + 1]
                    )

                # ---- BN1 stats allreduce ----
                st1 = small.tile([MID, 2], fp32)
                nc.vector.tensor_reduce(st1[:, 0:1], s1c[:], mybir.AxisListType.X, Alu.add)
                nc.vector.tensor_reduce(st1[:, 1:2], ss1c[:], mybir.AxisListType.X, Alu.add)
                nc.sync.dma_start(ar1_in[:], st1[:])
                nc.gpsimd.collective_compute(
                    "AllReduce", Alu.add, replica_groups=[list(range(NB))],
                    ins=[ar1_in[:]], outs=[ar1_out[:]],
                )
                st1r = small.tile([MID, 2], fp32)
                nc.sync.dma_start(st1r[:], ar1_out[:])

                def bn_coeffs(pool, stats, gb, nchan, tag):
                    # stats [C,2] (sum, sumsq) -> scale/bias [C,1] each
                    m = pool.tile([nchan, 4], fp32, tag=tag)
                    nc.vector.tensor_scalar_mul(m[:, 0:1], stats[:, 0:1], 1.0 / NSTAT)
                    nc.vector.tensor_scalar_mul(m[:, 1:2], stats[:, 1:2], 1.0 / NSTAT)
                    nc.vector.tensor_tensor(m[:, 2:3], m[:, 0:1], m[:, 0:1], Alu.mult)
                    nc.vector.tensor_tensor(m[:, 3:4], m[:, 1:2], m[:, 2:3], Alu.subtract)
                    epst = pool.tile([nchan, 1], fp32, tag=tag + "e")
                    nc.vector.memset(epst[:], EPS)
                    std = pool.tile([nchan, 1], fp32, tag=tag + "s")
                    nc.scalar.activation(std[:], m[:, 3:4], Act.Sqrt, bias=epst[:])
                    inv = pool.tile([nchan, 1], fp32, tag=tag + "i")
                    nc.vector.reciprocal(inv[:], std[:])
                    sc = pool.tile([nchan, 2], fp32, tag=tag + "c")
                    # scale = gamma * inv ; bias = beta - mean*scale
                    nc.vector.tensor_tensor(sc[:, 0:1], gb[:, 0:1], inv[:], Alu.mult)
                    tmpm = pool.tile([nchan, 1], fp32, tag=tag + "m")
                    nc.vector.tensor_tensor(tmpm[:], m[:, 0:1], sc[:, 0:1], Alu.mult)
                    nc.vector.tensor_tensor(sc[:, 1:2], gb[:, 1:2], tmpm[:], Alu.subtract)
                    return sc

                sc1 = bn_coeffs(small, st1r, g1b1, MID, "bn1")

                # ---- BN1 + SiLU into padded t1 ----
                t1pad = big.tile([MID, H1, H1], fp32)
                nc.vector.memset(t1pad[:], 0.0)
                for c in range(NCHUNK):
                    r0 = c * 8
                    nc.scalar.activation(
                        t1pad[:, 1 + r0 : 9 + r0, 1 : 1 + W],
                        y1[:, c * CHUNK : (c + 1) * CHUNK],
                        Act.Silu, bias=sc1[:, 1:2], scale=sc1[:, 0:1],
                    )

                # ---- conv2 (3x3, 64->100) + stats ----
                y2 = big.tile([NENC, PX], fp32)
                s2c = small.tile([NENC, NCHUNK], fp32)
                ss2c = small.tile([NENC, NCHUNK], fp32)
                for c in range(NCHUNK):
                    r0 = c * 8
                    pt = ps.tile([P, 2 * CHUNK], fp32, tag="wb", bufs=2, name="pt2")[:NENC, :CHUNK]
                    for tap in range(9):
                        dy, dx = tap // 3, tap % 3
                        nc.tensor.matmul(
                            pt[:],
                            w2t[:, tap * NENC : (tap + 1) * NENC],
                            t1pad[:, r0 + dy : r0 + dy + 8, dx : dx + W],
                            start=(tap == 0), stop=(tap == 8),
                        )
                    nc.scalar.activation(
                        y2[:, c * CHUNK : (c + 1) * CHUNK], pt[:], Act.Copy,
                        accum_out=s2c[:, c : c + 1],
                    )
                    sq = scratch.tile([NENC, CHUNK], fp32, tag="sq2")
                    nc.scalar.activation(
                        sq[:], pt[:], Act.Square, accum_out=ss2c[:, c : c + 1]
                    )

                # ---- BN2 stats allreduce ----
                st2 = small.tile([NENC, 2], fp32)
                nc.vector.tensor_reduce(st2[:, 0:1], s2c[:], mybir.AxisListType.X, Alu.add)
                nc.vector.tensor_reduce(st2[:, 1:2], ss2c[:], mybir.AxisListType.X, Alu.add)
                nc.sync.dma_start(ar2_in[:], st2[:])
                nc.gpsimd.collective_compute(
                    "AllReduce", Alu.add, replica_groups=[list(range(NB))],
                    ins=[ar2_in[:]], outs=[ar2_out[:]],
                )
                st2r = small.tile([NENC, 2], fp32)
                nc.sync.dma_start(st2r[:], ar2_out[:])
                sc2 = bn_coeffs(small, st2r, g2b2, NENC, "bn2")

                # ---- softmax numerators: e = exp(BN2(y2)) ----
                # BN output is ~N(0,1): exp without max-subtraction is safe in f32.
                esm = big.tile([NENC, PX], fp32, tag="ybuf", name="esm")
                for c in range(NCHUNK):
                    sl = slice(c * CHUNK, (c + 1) * CHUNK)
                    nc.scalar.activation(esm[:, sl], y2[:, sl], Act.Exp, bias=sc2[:, 1:2], scale=sc2[:, 0:1])

                # ---- softmax denominators + normalized weights ----
                r4 = big.tile([4, PX], fp32)
                wsm = y2  # y2 fully consumed by esm; reuse its storage
                for c in range(NCHUNK):
                    sl = slice(c * CHUNK, (c + 1) * CHUNK)
                    pd = ps.tile([P, 2 * CHUNK], fp32, tag="wb", bufs=2, name="pd")[:4, :CHUNK]
                    nc.tensor.matmul(pd[:], sel4[:], esm[:, sl], start=True, stop=True)
                    nc.vector.reciprocal(r4[:, sl], pd[:])
                for c in range(NCHUNK):
                    sl = slice(c * CHUNK, (c + 1) * CHUNK)
                    pr = ps.tile([P, 2 * CHUNK], fp32, tag="wb", bufs=2, name="pr")[:NENC, :CHUNK]
                    nc.tensor.matmul(pr[:], sel100[:], r4[:, sl], start=True, stop=True)
                    nc.vector.tensor_tensor(wsm[:, sl], esm[:, sl], pr[:], Alu.mult)

                # ---- reassembly ----
                for c in range(NCHUNK):
                    r0 = c * 8
                    acc_ps = ps.tile([P, 4 * CHUNK], fp32, tag="acc", bufs=1, name="acc_ps")

                    def emit_acc(group):
                        for s_, k_, tm_, sl_ in group:
                            nc.tensor.matmul(
                                acc_ps[:, s_ * CHUNK : (s_ + 1) * CHUNK], eye128[:],
                                tm_[:, sl_ * CHUNK : (sl_ + 1) * CHUNK],
                                start=(k_ == 0), stop=(k_ == 24), skip_group_check=True,
                            )

                    # software pipeline: round g emits broadcast+mult for group g
                    # and the accumulate matmuls for group g-1, so the in-order PE
                    # never stalls on the DVE behind an accumulate.
                    prev = None
                    for k in range(25):
                        dy, dx = k // 5, k % 5
                        xv = xpad[:, r0 + dy : r0 + dy + 8, dx : dx + W]
                        xv2 = bass.AP(xv.tensor, xv.offset, [xv.ap[0], [0, 2]] + list(xv.ap[1:]))
                        for sp in range(2):
                            wb = ps.tile([P, 2 * CHUNK], fp32, tag="wb", bufs=2, name="wb")
                            for sl in range(2):
                                ch = k * 4 + 2 * sp + sl
                                onehot = eye100[:, ch : ch + 1].to_broadcast((NENC, P))
                                nc.tensor.matmul(
                                    wb[:, sl * CHUNK : (sl + 1) * CHUNK], onehot,
                                    wsm[:, c * CHUNK : (c + 1) * CHUNK],
                                    start=True, stop=True,
                                )
                            tm = tmpp.tile([P, 2 * CHUNK], fp32, tag="tm", bufs=4)
                            nc.vector.tensor_tensor(tm[:], wb[:], xv2, Alu.mult)
                            cur = [(2 * sp, k, tm, 0), (2 * sp + 1, k, tm, 1)]
                            if prev is not None:
                                emit_acc(prev)
                            prev = cur
                    emit_acc(prev)
                    # de-interleave (s=(di,dj), i, j) -> out rows (2i+di), cols (2j+dj)
                    stage = tmpp.tile([P, 4 * CHUNK], fp32, tag="stg", bufs=2, name="stage")
                    for di in range(2):
                        stg_v = bass.AP(
                            stage.tensor, stage.offset + di * 128,
                            [stage.ap[0], [1, 2], [256, 8], [2, 64]],
                        )
                        acc_v = bass.AP(
                            acc_ps.tensor, acc_ps.offset + 2 * di * CHUNK,
                            [acc_ps.ap[0], [512, 2], [64, 8], [1, 64]],
                        )
                        nc.scalar.activation(stg_v, acc_v, Act.Copy)
                    nc.sync.dma_start(
                        out_d[:, c * 4 * CHUNK : (c + 1) * 4 * CHUNK], stage[:]
                    )

    nc.compile()
    return nc


def _prep_shared(comp_w, comp_g, comp_b, enc_w, enc_g, enc_b):
    w1t = np.ascontiguousarray(comp_w.reshape(MID, P).T)            # [128, 64]
    # w2t[tap] = enc_w[:, :, dy, dx].T  -> [64, 100] per tap, taps flattened
    w2t = np.ascontiguousarray(
        enc_w.transpose(2, 3, 1, 0).reshape(9, MID, NENC).transpose(1, 0, 2).reshape(MID, 9 * NENC)
    )
    g1b1 = np.stack([comp_g, comp_b], axis=1).astype(np.float32)    # [64, 2]
    g2b2 = np.stack([enc_g, enc_b], axis=1).astype(np.float32)      # [100, 2]
    ch = np.arange(NENC)
    sel4 = (ch[:, None] % 4 == np.arange(4)[None, :]).astype(np.float32)   # [100, 4]
    sel100 = np.ascontiguousarray(sel4.T)                                   # [4, 100]
    ones1 = np.ones((1, P), np.float32)
    eye100 = np.eye(NENC, dtype=np.float32)
    eye128 = np.eye(P, dtype=np.float32)
    return dict(w1t=w1t, w2t=w2t, g1b1=g1b1, g2b2=g2b2, sel4=sel4, sel100=sel100, ones1=ones1, eye100=eye100, eye128=eye128)


def kernel(x, comp_w, comp_g, comp_b, enc_w, enc_g, enc_b):
    from concourse.bass_utils import run_bass_kernel_spmd

    x = np.asarray(x, np.float32)
    shared = _prep_shared(
        np.asarray(comp_w, np.float32), np.asarray(comp_g, np.float32),
        np.asarray(comp_b, np.float32), np.asarray(enc_w, np.float32),
        np.asarray(enc_g, np.float32), np.asarray(enc_b, np.float32),
    )
    if "nc" not in _CACHE:
        _CACHE["nc"] = _build_program()
    nc = _CACHE["nc"]

    in_maps = []
    for i in range(NB):
        m = dict(shared)
        m["x"] = np.ascontiguousarray(x[i].reshape(P, PX))
        in_maps.append(m)

    res = run_bass_kernel_spmd(nc, in_maps, list(range(NB)))
    out = np.stack([res.results[i]["out"].reshape(P, HM, HM) for i in range(NB)])
    return out.astype(np.float32)



# revision 12
# speedup vs baseline: 3.6865x; 3.6865x over previous
"""CARAFE (content-aware reassembly of features) Trainium2 Bass kernel.

Problem (hardcoded shapes):
  x       [8, 128, 64, 64] f32
  comp_w  [64, 128, 1, 1]   1x1 conv -> BN(train stats) -> SiLU
  enc_w   [100, 64, 3, 3]   3x3 conv -> BN(train stats)
  pixel_shuffle(2) -> softmax over 25 taps -> weighted 5x5 (dilation 2)
  reassembly of nearest-upsampled x. Output [8, 128, 128, 128] f32.

Sharding: data-parallel over batch, 1 image per core on 8 cores.
BN batch stats are made exact with two tiny AllReduces (sum & sumsq).

Key layout trick: with output pixel (y,x) = (2i+di, 2j+dj) and tap (dy,dx),
the reassembly source is x[c, i+dy-2, j+dx-2] -- independent of (di,dj).
So everything runs at low resolution with shifted views of a zero-padded x;
the pixel-shuffle and nearest-upsample are folded into access patterns.

Performance structure (per core):
- all conv/broadcast/accumulate matmuls run at 1 cycle/row (fp16 or f32r
  inputs) instead of fp32's 4 cycles/row.
- softmax weights are computed in fp16. For 18 of the 25 taps ("DMA taps",
  grouped into 3 tribes of 6) the per-partition weight broadcast is done by
  DMA from a DRAM copy of the weights; the tap products then run on DVE in
  its 2x 16-bit mode. The remaining 7 taps ("PE taps") are broadcast by the
  PE into PSUM (f32) and their products run on the Pool engine, spreading
  the elementwise work across both vector engines.
- the 25-tap accumulation runs on the PE as identity matmuls into PSUM.
"""

import sys

import numpy as np

sys.path.insert(0, "/opt/trn_rl_repo")

P = 128          # partitions / input channels
MID = 64         # compressed channels
NENC = 100       # encoder output channels = 25 taps * 4 subpixels
H = W = 64
PX = H * W       # 4096 low-res pixels per image
HP = H + 4       # zero-padded (pad=2) low-res frame for 5x5 dil-2 taps
H1 = H + 2       # zero-padded (pad=1) frame for the 3x3 conv
HM = 2 * H       # 128 upsampled
OUT = HM * HM    # 16384 output pixels per image
NB = 8           # batch / cores
NSTAT = NB * PX  # BN normalization count (N*H*W)
EPS = 1e-5
CHUNK = 512      # free-dim chunk = 8 low-res rows
NCHUNK = PX // CHUNK

N_DMA_TAPS = 18                   # taps 0..17 broadcast via DMA, 18..24 via PE
TRIBE = 6                         # DMA taps per broadcast DMA
NTRIBE = N_DMA_TAPS // TRIBE
PE_TAPS = list(range(N_DMA_TAPS, 25))
# DMA-broadcast taps whose fp16 products run on the Pool engine (the rest,
# and all PE-tap products, run on DVE)
POOL_TAPS = {2, 4, 7, 10, 12, 15, 17}

_CACHE = {}


def _build_program():
    import concourse.bass as bass
    import concourse.mybir as mybir
    import concourse.tile as tile
    from concourse import bacc

    fp32 = mybir.dt.float32
    fp16 = mybir.dt.float16
    Alu = mybir.AluOpType
    Act = mybir.ActivationFunctionType

    nc = bacc.Bacc(None, num_devices=NB)

    with tile.TileContext(nc) as tc:
        with tc.tile_pool(name="dram", bufs=1, space="DRAM") as dram:
            # I/O
            x_d = dram.tile([P, PX], fp16, kind="ExternalInput", name="x16", uniquify=False)
            w1t_d = dram.tile([P, MID], fp16, kind="ExternalInput", name="w1t", uniquify=False)
            w2t_d = dram.tile([MID, 9 * NENC], fp16, kind="ExternalInput", name="w2t", uniquify=False)
            g1_d = dram.tile([MID, 2], fp32, kind="ExternalInput", name="g1b1", uniquify=False)
            g2_d = dram.tile([NENC, 2], fp32, kind="ExternalInput", name="g2b2", uniquify=False)
            sel4_d = dram.tile([NENC, 4], fp16, kind="ExternalInput", name="sel4", uniquify=False)
            sel100_d = dram.tile([4, NENC], fp16, kind="ExternalInput", name="sel100", uniquify=False)
            eyeh_d = dram.tile([NENC, NENC], fp16, kind="ExternalInput", name="eye100h", uniquify=False)
            e128h_d = dram.tile([P, P], fp16, kind="ExternalInput", name="eye128h", uniquify=False)
            out_d = dram.tile([P, OUT], fp32, kind="ExternalOutput", name="out", uniquify=False)
            # DRAM copy of fp16 softmax weights for the broadcast DMAs
            wsm_d = dram.tile([NENC, PX], fp16, name="wsm_dram")
            # collective bounce buffers (internal DRAM)
            ar1_in = dram.tile([2, MID], fp32, name="ar1_in")
            ar1_out = dram.tile([2, MID], fp32, name="ar1_out")
            ar2_in = dram.tile([2, NENC], fp32, name="ar2_in")
            ar2_out = dram.tile([2, NENC], fp32, name="ar2_out")

            with (
                tc.tile_pool(name="const", bufs=1) as const,
                tc.tile_pool(name="big", bufs=1) as big,
                tc.tile_pool(name="small", bufs=1) as small,
                tc.tile_pool(name="scratch", bufs=3) as scratch,
                tc.tile_pool(name="wbd", bufs=2) as wbdp,
                tc.tile_pool(name="tmp16", bufs=4) as tmp16,
                tc.tile_pool(name="tmpp", bufs=4) as tmpp,
                tc.tile_pool(name="stg", bufs=2) as stgp,
                tc.tile_pool(name="ps", bufs=2, space="PSUM") as ps,
            ):
                # ---- load constants ----
                w1t = const.tile([P, MID], fp16)
                nc.sync.dma_start(w1t[:], w1t_d[:])
                w2t = const.tile([MID, 9 * NENC], fp16)
                nc.sync.dma_start(w2t[:], w2t_d[:])
                g1b1 = const.tile([MID, 2], fp32)
                nc.sync.dma_start(g1b1[:], g1_d[:])
                g2b2 = const.tile([NENC, 2], fp32)
                nc.sync.dma_start(g2b2[:], g2_d[:])
                sel4 = const.tile([NENC, 4], fp16)
                nc.sync.dma_start(sel4[:], sel4_d[:])
                sel100 = const.tile([4, NENC], fp16)
                nc.sync.dma_start(sel100[:], sel100_d[:])
                eye100h = const.tile([NENC, NENC], fp16)
                nc.sync.dma_start(eye100h[:], eyeh_d[:])
                eye128h = const.tile([P, P], fp16)
                nc.sync.dma_start(eye128h[:], e128h_d[:])

                # ---- padded fp16 x ----
                xpad = big.tile([P, HP, HP], fp16)
                nc.vector.memset(xpad[:], 0.0)
                nc.sync.dma_start(
                    xpad[:, 2 : 2 + H, 2 : 2 + W],
                    x_d[:].rearrange("p (h w) -> p h w", h=H),
                )

                # ---- conv1 (1x1, 128->64) + stats ----
                y1 = big.tile([MID, PX], fp32, name="y1")
                s1c = small.tile([MID, NCHUNK], fp32)
                ss1c = small.tile([MID, NCHUNK], fp32)
                for c in range(NCHUNK):
                    r0 = c * 8
                    pt = ps.tile([P, 2 * CHUNK], fp32, tag="wb", bufs=2, name="pt1")[:MID, :CHUNK]
                    nc.tensor.matmul(
                        pt[:], w1t[:], xpad[:, 2 + r0 : 10 + r0, 2 : 2 + W], start=True, stop=True
                    )
                    nc.scalar.activation(
                        y1[:, c * CHUNK : (c + 1) * CHUNK], pt[:], Act.Copy,
                        accum_out=s1c[:, c : c + 1],
                    )
                    sq = scratch.tile([MID, CHUNK], fp32, tag="sq1")
                    nc.scalar.activation(
                        sq[:], pt[:], Act.Square, accum_out=ss1c[:, c : c + 1]
                    )

                # ---- BN1 stats allreduce ----
                st1 = small.tile([MID, 2], fp32)
                nc.vector.tensor_reduce(st1[:, 0:1], s1c[:], mybir.AxisListType.X, Alu.add)
                nc.vector.tensor_reduce(st1[:, 1:2], ss1c[:], mybir.AxisListType.X, Alu.add)
                nc.sync.dma_start(ar1_in[:], st1[:])
                nc.gpsimd.collective_compute(
                    "AllReduce", Alu.add, replica_groups=[list(range(NB))],
                    ins=[ar1_in[:]], outs=[ar1_out[:]],
                )
                st1r = small.tile([MID, 2], fp32)
                nc.sync.dma_start(st1r[:], ar1_out[:])

                def bn_coeffs(pool, stats, gb, nchan, tag):
                    # stats [C,2] (sum, sumsq) -> scale/bias [C,1] each
                    m = pool.tile([nchan, 4], fp32, tag=tag)
                    nc.vector.tensor_scalar_mul(m[:, 0:1], stats[:, 0:1], 1.0 / NSTAT)
                    nc.vector.tensor_scalar_mul(m[:, 1:2], stats[:, 1:2], 1.0 / NSTAT)
                    nc.vector.tensor_tensor(m[:, 2:3], m[:, 0:1], m[:, 0:1], Alu.mult)
                    nc.vector.tensor_tensor(m[:, 3:4], m[:, 1:2], m[:, 2:3], Alu.subtract)
                    epst = pool.tile([nchan, 1], fp32, tag=tag + "e")
                    nc.vector.memset(epst[:], EPS)
                    std = pool.tile([nchan, 1], fp32, tag=tag + "s")
                    nc.scalar.activation(std[:], m[:, 3:4], Act.Sqrt, bias=epst[:])
                    inv = pool.tile([nchan, 1], fp32, tag=tag + "i")
                    nc.vector.reciprocal(inv[:], std[:])
                    sc = pool.tile([nchan, 2], fp32, tag=tag + "c")
                    # scale = gamma * inv ; bias = beta - mean*scale
                    nc.vector.tensor_tensor(sc[:, 0:1], gb[:, 0:1], inv[:], Alu.mult)
                    tmpm = pool.tile([nchan, 1], fp32, tag=tag + "m")
                    nc.vector.tensor_tensor(tmpm[:], m[:, 0:1], sc[:, 0:1], Alu.mult)
                    nc.vector.tensor_tensor(sc[:, 1:2], gb[:, 1:2], tmpm[:], Alu.subtract)
                    return sc

                sc1 = bn_coeffs(small, st1r, g1b1, MID, "bn1")

                # ---- BN1 + SiLU into padded fp16 t1 ----
                t1pad = big.tile([MID, H1, H1], fp16)
                nc.vector.memset(t1pad[:], 0.0)
                for c in range(NCHUNK):
                    r0 = c * 8
                    nc.scalar.activation(
                        t1pad[:, 1 + r0 : 9 + r0, 1 : 1 + W],
                        y1[:, c * CHUNK : (c + 1) * CHUNK],
                        Act.Silu, bias=sc1[:, 1:2], scale=sc1[:, 0:1],
                    )

                # ---- conv2 (3x3, 64->100) + stats ----
                y2 = big.tile([NENC, PX], fp32)
                s2c = small.tile([NENC, NCHUNK], fp32)
                ss2c = small.tile([NENC, NCHUNK], fp32)
                for c in range(NCHUNK):
                    r0 = c * 8
                    pt = ps.tile([P, 2 * CHUNK], fp32, tag="wb", bufs=2, name="pt2")[:NENC, :CHUNK]
                    for tap in range(9):
                        dy, dx = tap // 3, tap % 3
                        nc.tensor.matmul(
                            pt[:],
                            w2t[:, tap * NENC : (tap + 1) * NENC],
                            t1pad[:, r0 + dy : r0 + dy + 8, dx : dx + W],
                            start=(tap == 0), stop=(tap == 8),
                        )
                    nc.scalar.activation(
                        y2[:, c * CHUNK : (c + 1) * CHUNK], pt[:], Act.Copy,
                        accum_out=s2c[:, c : c + 1],
                    )
                    sq = scratch.tile([NENC, CHUNK], fp32, tag="sq2")
                    nc.scalar.activation(
                        sq[:], pt[:], Act.Square, accum_out=ss2c[:, c : c + 1]
                    )

                # ---- BN2 stats allreduce ----
                st2 = small.tile([NENC, 2], fp32)
                nc.vector.tensor_reduce(st2[:, 0:1], s2c[:], mybir.AxisListType.X, Alu.add)
                nc.vector.tensor_reduce(st2[:, 1:2], ss2c[:], mybir.AxisListType.X, Alu.add)
                nc.sync.dma_start(ar2_in[:], st2[:])
                nc.gpsimd.collective_compute(
                    "AllReduce", Alu.add, replica_groups=[list(range(NB))],
                    ins=[ar2_in[:]], outs=[ar2_out[:]],
                )
                st2r = small.tile([NENC, 2], fp32)
                nc.sync.dma_start(st2r[:], ar2_out[:])
                sc2 = bn_coeffs(small, st2r, g2b2, NENC, "bn2")

                # ---- softmax numerators: e = exp(BN2(y2)), fp16 ----
                # BN output is ~N(0,1): exp without max-subtraction is safe.
                esm = big.tile([NENC, PX], fp16, name="esm")
                for c in range(NCHUNK):
                    sl = slice(c * CHUNK, (c + 1) * CHUNK)
                    nc.scalar.activation(esm[:, sl], y2[:, sl], Act.Exp, bias=sc2[:, 1:2], scale=sc2[:, 0:1])

                # ---- softmax denominators + normalized fp16 weights ----
                r4 = big.tile([4, PX], fp16)
                wsm = big.tile([NENC, PX], fp16, name="wsm")
                for c in range(NCHUNK):
                    sl = slice(c * CHUNK, (c + 1) * CHUNK)
                    pd = ps.tile([P, 2 * CHUNK], fp32, tag="wb", bufs=2, name="pd")[:4, :CHUNK]
                    nc.tensor.matmul(pd[:], sel4[:], esm[:, sl], start=True, stop=True)
                    with nc.allow_low_precision(reason="softmax reciprocal in fp16 is within tolerance"):
                        nc.vector.reciprocal(r4[:, sl], pd[:])
                for c in range(NCHUNK):
                    sl = slice(c * CHUNK, (c + 1) * CHUNK)
                    pr = ps.tile([P, 2 * CHUNK], fp32, tag="wb", bufs=2, name="pr")[:NENC, :CHUNK]
                    nc.tensor.matmul(pr[:], sel100[:], r4[:, sl], start=True, stop=True)
                    nc.vector.tensor_tensor(wsm[:, sl], esm[:, sl], pr[:], Alu.mult)
                    # stream the fp16 weights to DRAM for the broadcast DMAs
                    nc.sync.dma_start(wsm_d[:, sl], wsm[:, sl])

                # ---- reassembly ----
                # per chunk: 18 DMA-broadcast taps (products on DVE, fp16 2x)
                # + 7 PE-broadcast taps (products on Pool, f32), all taps
                # accumulated into one PSUM group via identity matmuls.
                for c in range(NCHUNK):
                    r0 = c * 8
                    acc_ps = ps.tile([P, 4 * CHUNK], fp32, tag="acc", bufs=1, name="acc_ps")

                    def emit_acc(group):
                        for s_, k_, rhs_, first_, last_ in group:
                            nc.tensor.matmul(
                                acc_ps[:, s_ * CHUNK : (s_ + 1) * CHUNK],
                                eye128h[:], rhs_,
                                start=first_, stop=last_, skip_group_check=True,
                            )

                    # broadcast DMAs for this chunk: one per tribe of 6 taps
                    wbd_tiles = []
                    for t in range(NTRIBE):
                        wbd = wbdp.tile([P, TRIBE * 4 * CHUNK], fp16, tag="wbd", name=f"wbd{t}")
                        src = bass.AP(
                            wsm_d.tensor,
                            wsm_d.offset + (t * TRIBE * 4) * PX + c * CHUNK,
                            [[0, P], [PX, TRIBE * 4], [1, CHUNK]],
                        )
                        nc.sync.dma_start(wbd[:], src)
                        wbd_tiles.append(wbd)

                    # interleave: process DMA taps, inserting one PE tap after
                    # every ~2.5 DMA taps so Pool/DVE/PE overlap
                    order = []
                    di, pi = 0, 0
                    pattern = [False, False, True, False, False, True, False, False, False, True]
                    # build order of 25 taps: PE taps spread among DMA taps
                    while di < N_DMA_TAPS or pi < len(PE_TAPS):
                        want_pe = pattern[(di + pi) % len(pattern)] if pi < len(PE_TAPS) else False
                        if want_pe or di >= N_DMA_TAPS:
                            order.append(("pe", PE_TAPS[pi])); pi += 1
                        else:
                            order.append(("dma", di)); di += 1

                    prev = []
                    nk = 0
                    for kind, k in order:
                        dy, dx = k // 5, k % 5
                        xv = xpad[:, r0 + dy : r0 + dy + 8, dx : dx + W]
                        if kind == "dma":
                            t, j = k // TRIBE, k % TRIBE
                            wbd = wbd_tiles[t]
                            # one product op over all 4 subpixels, x
                            # quadrupled via a stride-0 AP dim; fp16 end to end
                            xv4 = bass.AP(xv.tensor, xv.offset, [xv.ap[0], [0, 4]] + list(xv.ap[1:]))
                            tm = tmp16.tile([P, 4 * CHUNK], fp16, tag="tm16", name="tm16")
                            eng = nc.gpsimd if k in POOL_TAPS else nc.vector
                            eng.tensor_tensor(
                                tm[:], wbd[:, j * 4 * CHUNK : (j + 1) * 4 * CHUNK], xv4, Alu.mult
                            )
                            cur = [(s, k, tm[:, s * CHUNK : (s + 1) * CHUNK]) for s in range(4)]
                        else:
                            # PE broadcast (fp16 rhs -> f32 PSUM); products on
                            # DVE reading PSUM directly (GPSIMD cannot)
                            xv2 = bass.AP(xv.tensor, xv.offset, [xv.ap[0], [0, 2]] + list(xv.ap[1:]))
                            cur = []
                            for sp in range(2):
                                wb = ps.tile([P, 2 * CHUNK], fp32, tag="wb", bufs=2, name="wb")
                                for sl in range(2):
                                    ch = k * 4 + 2 * sp + sl
                                    onehot = eye100h[:, ch : ch + 1].to_broadcast((NENC, P))
                                    nc.tensor.matmul(
                                        wb[:, sl * CHUNK : (sl + 1) * CHUNK], onehot,
                                        wsm[:, c * CHUNK : (c + 1) * CHUNK],
                                        start=True, stop=True,
                                    )
                                tm = tmpp.tile([P, 2 * CHUNK], fp16, tag="tmp", name="tmp")
                                nc.vector.tensor_tensor(tm[:], wb[:], xv2, Alu.mult)
                                cur += [(2 * sp + sl, k, tm[:, sl * CHUNK : (sl + 1) * CHUNK]) for sl in range(2)]
                        # accumulate with one-tap lag so the in-order PE never
                        # stalls on a product that is still in flight
                        cur = [(s_, k_, rhs_, nk == 0, nk == 24) for (s_, k_, rhs_) in cur]
                        if prev:
                            emit_acc(prev)
                        prev = cur
                        nk += 1
                    emit_acc(prev)

                    # de-interleave (s=(di,dj), i, j) -> out rows (2i+di), cols (2j+dj)
                    stage = stgp.tile([P, 4 * CHUNK], fp32, tag="stg", name="stage")
                    for di in range(2):
                        stg_v = bass.AP(
                            stage.tensor, stage.offset + di * 128,
                            [stage.ap[0], [1, 2], [256, 8], [2, 64]],
                        )
                        acc_v = bass.AP(
                            acc_ps.tensor, acc_ps.offset + 2 * di * CHUNK,
                            [acc_ps.ap[0], [512, 2], [64, 8], [1, 64]],
                        )
                        nc.scalar.activation(stg_v, acc_v, Act.Copy)
                    nc.sync.dma_start(
                        out_d[:, c * 4 * CHUNK : (c + 1) * 4 * CHUNK], stage[:]
                    )

    nc.compile()
    return nc


def _prep_shared(comp_w, comp_g, comp_b, enc_w, enc_g, enc_b):
    w1t = np.ascontiguousarray(comp_w.reshape(MID, P).T).astype(np.float16)   # [128, 64]
    # w2t[tap] = enc_w[:, :, dy, dx].T  -> [64, 100] per tap, taps flattened
    w2t = np.ascontiguousarray(
        enc_w.transpose(2, 3, 1, 0).reshape(9, MID, NENC).transpose(1, 0, 2).reshape(MID, 9 * NENC)
    ).astype(np.float16)
    g1b1 = np.stack([comp_g, comp_b], axis=1).astype(np.float32)    # [64, 2]
    g2b2 = np.stack([enc_g, enc_b], axis=1).astype(np.float32)      # [100, 2]
    ch = np.arange(NENC)
    sel4 = (ch[:, None] % 4 == np.arange(4)[None, :]).astype(np.float16)   # [100, 4]
    sel100 = np.ascontiguousarray(sel4.T)                                   # [4, 100]
    eye100h = np.eye(NENC, dtype=np.float16)
    eye128h = np.eye(P, dtype=np.float16)
    return dict(
        w1t=w1t, w2t=w2t, g1b1=g1b1, g2b2=g2b2, sel4=sel4, sel100=sel100,
        eye100h=eye100h, eye128h=eye128h,
    )


def _make_in_maps(x, shared):
    in_maps = []
    for i in range(NB):
        m = dict(shared)
        m["x16"] = np.ascontiguousarray(x[i].reshape(P, PX)).astype(np.float16)
        in_maps.append(m)
    return in_maps


def kernel(x, comp_w, comp_g, comp_b, enc_w, enc_g, enc_b):
    from concourse.bass_utils import run_bass_kernel_spmd

    x = np.asarray(x, np.float32)
    shared = _prep_shared(
        np.asarray(comp_w, np.float32), np.asarray(comp_g, np.float32),
        np.asarray(comp_b, np.float32), np.asarray(enc_w, np.float32),
        np.asarray(enc_g, np.float32), np.asarray(enc_b, np.float32),
    )
    if "nc" not in _CACHE:
        _CACHE["nc"] = _build_program()
    nc = _CACHE["nc"]

    res = run_bass_kernel_spmd(nc, _make_in_maps(x, shared), list(range(NB)))
    out = np.stack([res.results[i]["out"].reshape(P, HM, HM) for i in range(NB)])
    return out.astype(np.float32)
